# revision 19
# baseline (speedup 1.0000x reference)
"""Sliding-window causal GQA self-attention kernel for 8 Trainium2 NeuronCores.

Sharding: core c -> (batch b = c//4, kv-head g = c%4, q-heads 4g..4g+3).
Each core computes its 4 q-heads' attention and a partial output projection
(y_heads @ Wo[rows]); the host sums the 4 partials per batch.

On-chip layout is feature-major ("transposed"): activations live as
[features, tokens] tiles so every matmul contracts over the partition dim.
Scores are computed k-major (sT[k, q]); softmax needs no max-subtraction
because q/k are rms-normalized (|score| <= 8). All 4 q-heads share one kv
head (GQA rep=4), so QK and PV matmuls batch two heads per instruction
(moving free dim 512 = 2 heads x 256 q) against a single stationary.
The softmax denominator comes from 64 ones-columns appended to V: PV output
rows 64:128 all hold the denominator, so one wide ScalarE reciprocal
produces a broadcast-ready [64, 512] scale. Causal and sliding-window masks
are added in PSUM by triangular-counting matmuls. All matmuls use the
float32r dtype view (full-rate fp32 at free-dim >= 256).
"""

import numpy as np

import concourse.bass as bass
import concourse.mybir as mybir
import concourse.tile as tile
from concourse.bass import ds, ts

F32 = mybir.dt.float32
F32R = mybir.dt.float32r
BF16 = mybir.dt.bfloat16
AF = mybir.ActivationFunctionType

B, T, NE = 2, 2048, 1024
NH, NKV, HD = 16, 4, 64
GC = 32
WIN = 1024
EPS = 1e-6
BIG = 1.0e9
NCORES = 8
QB = 256          # q-block (per head; 2 heads -> 512 moving free dim)
NQB = T // QB     # 8
NKB = T // 128    # 16 k-blocks
SCALE = 1.0 / 8.0  # 1/sqrt(HD)


def _r(x):
    return x.bitcast(F32R)


def _dma_r(nc, dst, src):
    nc.sync.dma_start(_r(dst), _r(src))


def _build_nc():
    nc = bass.Bass(trn_type="TRN2", target_bir_lowering=False)

    d = {}
    for name, shape in [
        ("xT", (NE, T)), ("ve", (128, NKB * HD)),
        ("cos4", (128, T)), ("sin4", (128, T)),
        ("coskv", (128, T)), ("sinkv", (128, T)),
        ("wq", (NE, 256)), ("wkv", (NE, 128)), ("wg", (GC, 1)),
        ("wo", (256, NE)),
        ("pswq", (128, 128)),
        ("bdq", (128, 2)), ("bdk", (128, 1)),
        ("e2sel", (2, 128)), ("ident", (128, 128)),
        ("onesrow", (1, T)),
    ]:
        d[name] = nc.dram_tensor(name, list(shape), F32, kind="ExternalInput")
    for name, shape in [
        ("triA", (128, 128)), ("triA2", (128, 128)),
        ("bc0", (128, 512)), ("bc1", (128, 512)),
        ("bw0", (128, 512)), ("bw1", (128, 512)),
        ("onesv", (128, 64)),
    ]:
        d[name] = nc.dram_tensor(name, list(shape), BF16, kind="ExternalInput")
    out_d = nc.dram_tensor("out", [T, NE], F32, kind="ExternalOutput")

    with tile.TileContext(nc) as tc:
        with (
            nc.allow_low_precision(reason="float32r views of fp32 data"),
            tc.tile_pool(name="persist", bufs=1) as pp,
            tc.tile_pool(name="smalls", bufs=4) as sm,
        ):
            # ---- persistent tiles ----
            # qcat[f, h, t]: 64 features x 4 q-heads x T tokens, rms-scaled
            qcat = pp.tile([64, NH // NKV, T], BF16, tag="qcat")
            # khat: k-hat (roped+rms) bf16; vraw: raw v fp32
            khat = pp.tile([64, T], BF16, tag="khat")
            vraw = pp.tile([64, T], F32, tag="vraw")
            # vaug[kb]: [128 tokens, 64 v-features + 64 ones]
            vaug = [pp.tile([128, 128], BF16, tag=f"vaug{k}", name=f"vaug{k}")
                    for k in range(NKB)]
            cst = {}
            for nm, shp in [("e2sel", [2, 128]), ("ident", [128, 128])]:
                cst[nm] = pp.tile(shp, F32, tag=nm, name=nm)
            _dma_r(nc, cst["e2sel"][:], d["e2sel"][:])
            nc.sync.dma_start(cst["ident"][:], d["ident"][:])
            eps_sb = pp.tile([128, 1], F32, tag="eps")
            nc.vector.memset(eps_sb[:], EPS)
            for kb in range(NKB):
                nc.sync.dma_start(vaug[kb][:, HD:128], d["onesv"][:])

            # =================================================================
            # Phase A: projections + rope + rmsnorm + vaug build
            # =================================================================
            with (
                tc.tile_pool(name="xp", bufs=1) as xp,
                tc.tile_pool(name="work", bufs=2) as wk,
                tc.tile_pool(name="trig", bufs=1) as trg,
                tc.tile_pool(name="pj_ps", bufs=2, space="PSUM") as pjp,
                tc.tile_pool(name="sw_ps", bufs=2, space="PSUM") as swp,
                tc.tile_pool(name="aux_ps", bufs=1, space="PSUM") as axp,
            ):
                xsb = [xp.tile([128, T], F32, tag=f"x{e}", name=f"x{e}") for e in range(8)]
                wq_sb = [xp.tile([128, 256], F32, tag=f"wq{e}", name=f"wqs{e}")
                         for e in range(8)]
                wkv_sb = [xp.tile([128, 128], F32, tag=f"wkv{e}", name=f"wkvs{e}")
                          for e in range(8)]
                for e in range(8):
                    _dma_r(nc, xsb[e][:], d["xT"][ds(128 * e, 128), :])
                    _dma_r(nc, wq_sb[e][:], d["wq"][ds(128 * e, 128), :])
                    _dma_r(nc, wkv_sb[e][:], d["wkv"][ds(128 * e, 128), :])
                wg_sb = sm.tile([GC, 1], F32, tag="wg")
                _dma_r(nc, wg_sb[:], d["wg"][:])
                ve_sb = xp.tile([128, NKB, HD], F32, tag="ve")
                _dma_r(nc, ve_sb[:], d["ve"][:, :])
                aux = {}
                for nm, shp in [("pswq", [128, 128]),
                                ("bdq", [128, 2]), ("bdk", [128, 1])]:
                    aux[nm] = xp.tile(shp, F32, tag=nm, name=f"aux_{nm}")
                    _dma_r(nc, aux[nm][:], d[nm][:])

                # gate: u = x[:, :GC] @ wg ; g2 = 2*sigmoid(u) token-major
                gate_ps = axp.tile([128, NKB], F32, tag="aux")
                for kb in range(NKB):
                    nc.tensor.matmul(
                        gate_ps[:, ds(kb, 1)],
                        xsb[0][0:GC, ts(kb, 128)], wg_sb[:],
                        start=True, stop=True)
                g2 = xp.tile([128, NKB], F32, tag="g2")
                nc.scalar.activation(g2[:], gate_ps[:], AF.Exp, scale=-1.0)
                nc.vector.tensor_scalar_add(g2[:], g2[:], 1.0)
                nc.vector.reciprocal(g2[:], g2[:])
                nc.scalar.mul(g2[:], g2[:], 2.0)

                def proj_part1(widx, w_tiles, mcols, psw, cos_t, sin_t,
                               bd, nh):
                    """x @ W -> roped [128, T] (in place) + rms stats in PSUM."""
                    raw = wk.tile([128, T], F32, tag="w0")
                    for nchk in range(4):
                        cols = ds(512 * nchk, 512)
                        ps = pjp.tile([128, 512], F32, tag="pj")
                        for e in range(8):
                            nc.tensor.matmul(
                                ps[:], _r(w_tiles[e][:, mcols]),
                                _r(xsb[e][:, cols]),
                                start=(e == 0), stop=(e == 7))
                        nc.any.tensor_copy(_r(raw[:, cols]), ps[:])
                    msps = axp.tile([nh, T], F32, tag="aux")
                    for nchk in range(4):
                        cols = ds(512 * nchk, 512)
                        sw = swp.tile([128, 512], F32, tag="sw")
                        nc.tensor.matmul(sw[:], _r(psw[:]), _r(raw[:, cols]),
                                         start=True, stop=True)
                        t1c = wk.tile([128, 512], F32, tag="w1")
                        nc.vector.tensor_mul(_r(t1c[:]), raw[:, cols],
                                             cos_t[:, cols])
                        tm2 = wk.tile([128, 512], F32, tag="w2")
                        nc.vector.tensor_mul(tm2[:], sw[:], sin_t[:, cols])
                        nc.vector.tensor_add(_r(raw[:, cols]), t1c[:], tm2[:])
                        sqc = wk.tile([128, 512], F32, tag="w3")
                        nc.scalar.activation(_r(sqc[:]), raw[:, cols],
                                             AF.Square)
                        nc.tensor.matmul(msps[:, cols], _r(bd[:, 0:nh]),
                                         _r(sqc[:]), start=True, stop=True)
                    return raw, msps

                def proj_rs(widx, msps, nh):
                    """rsqrt(mean-sq) -> rs [2, T] (row 1 stays 1 for nh=1)."""
                    lnm = sm.tile([2, T], F32, tag="lnm", bufs=1,
                                  name=f"lnm{widx}")
                    nc.scalar.activation(lnm[0:nh, :], msps[:], AF.Ln,
                                         scale=1.0 / HD, bias=eps_sb[0:nh, :])
                    rs = sm.tile([2, T], F32, tag="rs", bufs=2,
                                 name=f"rs{widx}")
                    if nh == 1:
                        _dma_r(nc, rs[0:2, :],
                               d["onesrow"][:, :].to_broadcast([2, T]))
                    nc.scalar.activation(_r(rs[0:nh, :]), lnm[0:nh, :], AF.Exp,
                                         scale=-0.5)
                    return rs

                def rsb_chunk(rs, nchk):
                    cols = ds(512 * nchk, 512)
                    rsb = swp.tile([128, 512], F32, tag="sw")
                    nc.tensor.matmul(rsb[:], _r(cst["e2sel"][:]),
                                     _r(rs[0:2, cols]), start=True, stop=True)
                    return rsb

                # --- part1 of kv, then q0 (keeps PE busy during kv stats) ---
                cos_kv = trg.tile([128, T], F32, tag="tc")
                sin_kv = trg.tile([128, T], F32, tag="tsn")
                nc.sync.dma_start(cos_kv[:], d["coskv"][:])
                nc.sync.dma_start(sin_kv[:], d["sinkv"][:])
                ropedkv, msps_kv = proj_part1(2, wkv_sb, ds(0, 128),
                                              aux["pswq"], cos_kv, sin_kv,
                                              aux["bdk"], 1)
                cos_q = trg.tile([128, T], F32, tag="tcq")
                sin_q = trg.tile([128, T], F32, tag="tsnq")
                nc.sync.dma_start(cos_q[:], d["cos4"][:])
                nc.sync.dma_start(sin_q[:], d["sin4"][:])
                roped_q = [None, None]
                roped_q[0], msps_q0 = proj_part1(0, wq_sb, ds(0, 128),
                                                 aux["pswq"], cos_q, sin_q,
                                                 aux["bdq"], 2)
                # --- kv finish: khat (bf16) + vraw ---
                rs_kv = proj_rs(2, msps_kv, 1)
                for nchk in range(4):
                    cols = ds(512 * nchk, 512)
                    rsb = rsb_chunk(rs_kv, nchk)
                    nc.vector.tensor_mul(khat[:, cols], ropedkv[0:64, cols],
                                         rsb[0:64, :])
                    nc.vector.tensor_mul(_r(vraw[:, cols]),
                                         ropedkv[64:128, cols],
                                         rsb[64:128, :])
                # --- q1 part1 ---
                roped_q[1], msps_q1 = proj_part1(1, wq_sb, ds(128, 128),
                                                 aux["pswq"], cos_q, sin_q,
                                                 aux["bdq"], 2)
                # --- vaug build (PE transposes fill the q-stats junctures) ---
                for kb in range(NKB):
                    vt = pjp.tile([128, HD], F32, tag="pj")
                    nc.tensor.transpose(vt[:], vraw[:, ts(kb, 128)],
                                        cst["ident"][0:64, 0:64])
                    gv = sm.tile([128, HD], F32, tag="gv")
                    nc.vector.tensor_scalar_mul(gv[:], ve_sb[:, kb, :],
                                                g2[:, ds(kb, 1)])
                    nc.vector.tensor_add(vaug[kb][:, 0:HD], gv[:], vt[:])
                # --- q finishes -> qcat (bf16) ---
                for i, msps_qi in ((0, msps_q0), (1, msps_q1)):
                    rs = proj_rs(i, msps_qi, 2)
                    for nchk in range(4):
                        cols = ds(512 * nchk, 512)
                        rsb = rsb_chunk(rs, nchk)
                        for h in range(2):
                            rows = ds(64 * h, 64)
                            nc.vector.tensor_mul(
                                qcat[:, 2 * i + h, cols],
                                roped_q[i][rows, cols], rsb[rows, :])

            # =================================================================
            # Phase B: attention + output projection, software-pipelined:
            # block qb emits QK+exp(qb) interleaved with PV(qb-1), then
            # recip/normalize/outproj(qb-1). The in-order PE never waits on
            # ScalarE exp: PV and outproj of the previous block fill the
            # stream while exp of this block lags behind QK.
            # =================================================================
            with (
                tc.tile_pool(name="pb", bufs=1) as pb,
                tc.tile_pool(name="sc_ps", bufs=2, space="PSUM") as scp,
                tc.tile_pool(name="yt_ps", bufs=1, space="PSUM") as ytp,
                tc.tile_pool(name="po_ps", bufs=2, space="PSUM") as pop,
                tc.tile_pool(name="et", bufs=13) as etp,
                tc.tile_pool(name="ri", bufs=2) as rip,
                tc.tile_pool(name="stage", bufs=4) as stg,
            ):
                ytall = [pb.tile([128, T], F32, tag=f"ytall{i}",
                                 name=f"ytall{i}") for i in range(2)]
                wo_sb = [[pb.tile([128, 512], F32, tag=f"wo{i}{n}",
                                  name=f"wo{i}{n}")
                          for n in range(2)] for i in range(2)]
                for nm in ("triA", "triA2", "bc0", "bc1", "bw0", "bw1"):
                    shp = [128, 128] if nm.startswith("tri") else [128, 512]
                    cst[nm] = pb.tile(shp, BF16, tag=nm, name=f"pb_{nm}")
                    nc.sync.dma_start(cst[nm][:], d[nm][:])
                for i in range(2):
                    for n in range(2):
                        _dma_r(nc, wo_sb[i][n][:],
                               d["wo"][ds(128 * i, 128), ds(512 * n, 512)])

                def kbs_of(qb):
                    return list(range(max(0, 2 * qb - 8), 2 * qb + 2))

                ets = {}       # (qb, kb) -> et tile
                yts_cur = [None]  # yts tile of the in-flight PV block

                def emit_qk2(qb, kbA, kbB):
                    """QK + masks + exp for a pair of k-blocks. Stationaries
                    (khat slice, tri matrix) are loaded once and reused by
                    setting ldweights=False on the following matmuls."""
                    if kbA == 2 * qb:
                        masks = (cst["triA"], (cst["bc0"], cst["bc1"]))
                    elif kbA == 2 * qb - 8:
                        masks = (cst["triA2"], (cst["bw0"], cst["bw1"]))
                    else:
                        masks = None
                    scs = {}
                    for kb in (kbA, kbB):
                        sc = scp.tile([128, 1024], F32, tag="score",
                                      name=f"sc{qb}_{kb}")
                        scs[kb] = sc
                        nc.tensor.ldweights(khat[:, ts(kb, 128)])
                        for pair in range(2):
                            mm = nc.tensor.matmul(
                                sc[:, ds(512 * pair, 512)],
                                khat[:, ts(kb, 128)],
                                qcat[:, ds(2 * pair, 2), ds(QB * qb, QB)],
                                start=True, stop=(masks is None))
                            mm.ins.ldweights = False
                    if masks is not None:
                        tri, bcs = masks
                        nc.tensor.ldweights(tri[:])
                        for kb, bc in zip((kbA, kbB), bcs):
                            for pair in range(2):
                                mm = nc.tensor.matmul(
                                    scs[kb][:, ds(512 * pair, 512)],
                                    tri[:], bc[:], start=False, stop=True)
                                mm.ins.ldweights = False
                    for kb in (kbA, kbB):
                        et = etp.tile([128, 1024], BF16, tag="et",
                                      name=f"et{qb}_{kb}")
                        nc.scalar.activation(et[:], scs[kb][:], AF.Exp,
                                             scale=SCALE)
                        ets[(qb, kb)] = et

                def emit_pv2(qb, kbA, kbB, first, last):
                    if first:
                        yts_cur[0] = ytp.tile([128, 1024], F32, tag="yt",
                                              name=f"yt{qb}")
                    for j, kb in enumerate((kbA, kbB)):
                        et = ets.pop((qb, kb))
                        nc.tensor.ldweights(vaug[kb][:])
                        for pair in range(2):
                            mm = nc.tensor.matmul(
                                yts_cur[0][:, ds(512 * pair, 512)],
                                vaug[kb][:], et[:, ds(512 * pair, 512)],
                                start=first and j == 0,
                                stop=last and j == 1)
                            mm.ins.ldweights = False

                def emit_recip(qb):
                    """1/denominator via exp(-ln d) on ScalarE."""
                    yts = yts_cur[0]
                    lnd = rip.tile([64, 1024], F32, tag="lnd")
                    nc.scalar.activation(lnd[:], yts[64:128, :], AF.Ln)
                    rinv = rip.tile([64, 1024], F32, tag="ri")
                    nc.scalar.activation(_r(rinv[:]), lnd[:], AF.Exp,
                                         scale=-1.0)
                    return yts, rinv

                def emit_norm(qb, yts, rinv):
                    qsl = ds(QB * qb, QB)
                    for pair in range(2):
                        for h in range(2):
                            nc.vector.tensor_mul(
                                _r(ytall[pair][ds(64 * h, 64), qsl]),
                                yts[0:HD, ds(512 * pair + QB * h, QB)],
                                rinv[:, ds(512 * pair + QB * h, QB)])

                def emit_outproj(tt):
                    for nn in range(2):
                        po = pop.tile([128, 512], F32, tag="po")
                        for i in range(2):
                            nc.tensor.matmul(
                                po[:], _r(ytall[i][:, ts(tt, 128)]),
                                _r(wo_sb[i][nn][:]),
                                start=(i == 0), stop=(i == 1))
                        osb = stg.tile([128, 512], F32, tag="osb")
                        nc.vector.tensor_copy(_r(osb[:]), po[:])
                        nc.sync.dma_start(
                            out_d[ts(tt, 128), ds(512 * nn, 512)], osb[:])

                for qb in range(NQB + 1):
                    cur = kbs_of(qb) if qb < NQB else []
                    prev = kbs_of(qb - 1) if qb > 0 else []
                    groups = [(cur[j], cur[j + 1])
                              for j in range(0, len(cur), 2)]
                    # first QK group, then ALL PVs of the previous block
                    # (their ets are ready - pure dense PE work)
                    if groups:
                        emit_qk2(qb, *groups[0])
                    for j in range(0, len(prev), 2):
                        emit_pv2(qb - 1, prev[j], prev[j + 1],
                                 j == 0, j + 2 >= len(prev))
                    # recip of prev starts on ScalarE while the remaining QK
                    # groups run; norm + outproj interleave into the QK tail
                    # so the PE never queues behind the recip chain.
                    tail = []
                    if prev:
                        yts_p, rinv_p = emit_recip(qb - 1)
                        tail = [lambda: emit_norm(qb - 1, yts_p, rinv_p),
                                lambda: emit_outproj(2 * (qb - 1)),
                                lambda: emit_outproj(2 * (qb - 1) + 1)]
                    k = 0
                    for g in groups[1:]:
                        emit_qk2(qb, *g)
                        if k < len(tail):
                            tail[k]()
                            k += 1
                    while k < len(tail):
                        tail[k]()
                        k += 1

    return nc


# ---------------------------------------------------------------------------
# walrus workaround: this build rejects >1 sync-wait on CTRL-class ops
# (e.g. the Tile tail Drain). Move excess waits onto NOPs inserted before.
# ---------------------------------------------------------------------------
_CTRL_TYPES = (mybir.InstDrain, mybir.InstNoOp, mybir.InstEventSemaphore)


def _split_excess_waits(nc, limit=1):
    for fn in nc.m.functions:
        for bb in fn.blocks:
            out, changed = [], False
            for inst in bb.instructions:
                si = inst.sync_info
                waits = list(si.on_wait) if si is not None and si.on_wait else []
                if len(waits) > limit:
                    extra, keep = waits[:-limit], waits[-limit:]
                    while extra:
                        chunk, extra = extra[:limit], extra[limit:]
                        nop = mybir.InstNoOp(
                            name=f"{inst.name}-wsplit{len(out)}", ins=[],
                            outs=[])
                        nop.engine = inst.engine
                        nop.sync_info = mybir.SyncInfo(on_wait=chunk,
                                                       on_update=[])
                        out.append(nop)
                    si.on_wait = keep
                    inst.sync_info = si
                    changed = True
                out.append(inst)
            if changed:
                bb.instructions = out


# ---------------------------------------------------------------------------
# Host-side constants (shared by all cores)
# ---------------------------------------------------------------------------
def _host_constants():
    import ml_dtypes
    bf16 = ml_dtypes.bfloat16
    c = {}
    m = np.arange(128)[:, None]
    i = np.arange(QB)[None, :]
    c["triA"] = (m <= np.arange(128)[None, :]).astype(bf16)
    c["triA2"] = (m >= np.arange(128)[None, :]).astype(bf16)
    bc0 = np.where(m > i, -BIG, 0.0).astype(np.float32)
    bc1 = np.where(m > i - 128, -BIG, 0.0).astype(np.float32)
    bw0 = np.where(m < i, -BIG, 0.0).astype(np.float32)
    bw1 = np.where(m + 128 < i, -BIG, 0.0).astype(np.float32)
    for nm, base in (("bc0", bc0), ("bc1", bc1), ("bw0", bw0), ("bw1", bw1)):
        c[nm] = np.tile(base, (1, 2)).astype(bf16)
    sw = np.zeros((128, 128), np.float32)            # pswq[f, m]=1 iff f=sig(m)
    for mm in range(128):
        f = mm + 32 if (mm % 64) < 32 else mm - 32
        sw[f, mm] = 1.0
    c["pswq"] = sw
    bdq = np.zeros((128, 2), np.float32)
    bdq[0:64, 0] = 1.0
    bdq[64:128, 1] = 1.0
    c["bdq"] = bdq
    bdk = np.zeros((128, 1), np.float32)
    bdk[0:64, 0] = 1.0
    c["bdk"] = bdk
    e2 = np.zeros((2, 128), np.float32)
    e2[0, 0:64] = 1.0
    e2[1, 64:128] = 1.0
    c["e2sel"] = e2
    c["ident"] = np.eye(128, dtype=np.float32)
    c["onesrow"] = np.ones((1, T), np.float32)
    c["onesv"] = np.ones((128, 64), bf16)
    return c


def _trig(cos_b, sin_b):
    """cos_b/sin_b: [T, HD//2] -> the four [128, T] rope coefficient maps."""
    ct = np.ascontiguousarray(cos_b.T)               # [32, T]
    st = np.ascontiguousarray(sin_b.T)
    cos4 = np.tile(ct, (4, 1)).astype(np.float32)    # [c;c;c;c]
    sin4 = np.tile(np.concatenate([st, -st], 0), (2, 1)).astype(np.float32)
    coskv = np.concatenate([ct, ct, np.ones((64, T), np.float32)], 0)
    sinkv = np.concatenate([st, -st, np.zeros((64, T), np.float32)], 0)
    return cos4, sin4, coskv.astype(np.float32), sinkv.astype(np.float32)


# ---------------------------------------------------------------------------
# Cached PJRT runner (compile once per process)
# ---------------------------------------------------------------------------
_RUNNER = None


def _get_runner():
    global _RUNNER
    if _RUNNER is not None:
        return _RUNNER
    import jax
    from jax.experimental.shard_map import shard_map
    from jax.sharding import Mesh, PartitionSpec
    from concourse.bass2jax import (_bass_exec_p, install_neuronx_cc_hook,
                                    partition_id_tensor)

    nc = _build_nc()
    _split_excess_waits(nc)
    install_neuronx_cc_hook()

    pid_name = (nc.partition_id_tensor.name
                if nc.partition_id_tensor is not None else None)
    in_names, out_names, out_avals, zero_outs = [], [], [], []
    for alloc in nc.m.functions[0].allocations:
        if not isinstance(alloc, mybir.MemoryLocationSet):
            continue
        name = alloc.memorylocations[0].name
        if alloc.kind == "ExternalInput":
            if name == pid_name:
                continue
            in_names.append(name)
        elif alloc.kind == "ExternalOutput":
            np_dt = mybir.dt.np(alloc.dtype)
            out_names.append(name)
            out_avals.append(
                jax.core.ShapedArray(tuple(alloc.tensor_shape), np_dt))
            zero_outs.append(
                np.zeros(tuple(alloc.tensor_shape), np_dt))

    def _body(*args):
        operands = list(args)
        if pid_name is not None:
            operands.append(partition_id_tensor())
        outs = _bass_exec_p.bind(
            *operands,
            out_avals=tuple(out_avals),
            in_names=(tuple(in_names) + tuple(out_names)
                      + ((pid_name,) if pid_name else ())),
            out_names=tuple(out_names),
            lowering_input_output_aliases=(),
            sim_require_finite=True,
            sim_require_nnan=True,
            nc=nc,
        )
        return tuple(outs)

    devices = jax.devices()[:NCORES]
    mesh = Mesh(np.asarray(devices), ("core",))
    n_args = len(in_names) + len(out_names)
    sharded = jax.jit(
        shard_map(_body, mesh=mesh,
                  in_specs=(PartitionSpec("core"),) * n_args,
                  out_specs=(PartitionSpec("core"),) * len(out_names),
                  check_rep=False),
        keep_unused=True,
    )

    def run(in_maps):
        concat_in = [
            np.concatenate([in_maps[c][nm] for c in range(NCORES)], axis=0)
            for nm in in_names
        ]
        concat_zero = [
            np.zeros((NCORES * z.shape[0], *z.shape[1:]), z.dtype)
            for z in zero_outs
        ]
        outs = sharded(*concat_in, *concat_zero)
        res = []
        for c in range(NCORES):
            res.append({
                nm: np.asarray(outs[i]).reshape(NCORES, *out_avals[i].shape)[c]
                for i, nm in enumerate(out_names)
            })
        return res

    _RUNNER = {"run": run, "sharded": sharded, "in_names": in_names,
               "out_names": out_names, "out_avals": out_avals,
               "zero_outs": zero_outs, "nc": nc, "mesh": mesh}
    return _RUNNER


def _make_in_maps(x, ve, cos, sin, Wq, Wk, Wv, Wo, Wg):
    cstc = _host_constants()
    in_maps = []
    for c in range(NCORES):
        b, g = c // 4, c % 4
        cos4, sin4, coskv, sinkv = _trig(np.asarray(cos[b]),
                                         np.asarray(sin[b]))
        m = {
            "xT": np.ascontiguousarray(np.asarray(x[b]).T),
            "ve": np.ascontiguousarray(
                np.asarray(ve[b])[:, HD * g:HD * (g + 1)]
                .reshape(NKB, 128, HD).transpose(1, 0, 2).reshape(128, -1)),
            "cos4": cos4, "sin4": sin4, "coskv": coskv, "sinkv": sinkv,
            "wq": np.ascontiguousarray(Wq[:, 256 * g:256 * (g + 1)]),
            "wkv": np.ascontiguousarray(
                np.concatenate([Wk[:, HD * g:HD * (g + 1)],
                                Wv[:, HD * g:HD * (g + 1)]], axis=1)),
            "wg": np.ascontiguousarray(Wg[:, g:g + 1]),
            "wo": np.ascontiguousarray(Wo[256 * g:256 * (g + 1), :]),
        }
        m = {k: np.asarray(v, np.float32) for k, v in m.items()}
        m.update(cstc)
        in_maps.append(m)
    return in_maps


def kernel(x, ve, cos, sin, Wq, Wk, Wv, Wo, Wg, window_size):
    assert int(window_size) == WIN, f"kernel hardcodes window={WIN}"
    x, ve, cos, sin = (np.asarray(a, np.float32) for a in (x, ve, cos, sin))
    Wq, Wk, Wv, Wo, Wg = (np.asarray(a, np.float32)
                          for a in (Wq, Wk, Wv, Wo, Wg))
    runner = _get_runner()
    in_maps = _make_in_maps(x, ve, cos, sin, Wq, Wk, Wv, Wo, Wg)
    res = runner["run"](in_maps)
    out = np.zeros((B, T, NE), np.float32)
    for c in range(NCORES):
        out[c // 4] += res[c]["out"]
    return out


# revision 20
# speedup vs baseline: 1.0473x; 1.0473x over previous
"""Sliding-window causal GQA self-attention kernel for 8 Trainium2 NeuronCores.

Sharding: core c -> (batch b = c//4, kv-head g = c%4, q-heads 4g..4g+3).
Each core computes its 4 q-heads' attention and a partial output projection
(y_heads @ Wo[rows]); the host sums the 4 partials per batch.

On-chip layout is feature-major ("transposed"): activations live as
[features, tokens] tiles so every matmul contracts over the partition dim.
Scores are computed k-major (sT[k, q]); softmax needs no max-subtraction
because q/k are rms-normalized (|score| <= 8). All 4 q-heads share one kv
head (GQA rep=4), so QK and PV matmuls batch two heads per instruction
(moving free dim 512 = 2 heads x 256 q) against a single stationary.
The softmax denominator comes from 64 ones-columns appended to V: PV output
rows 64:128 all hold the denominator, so one wide ScalarE reciprocal
produces a broadcast-ready [64, 512] scale. Causal and sliding-window masks
are added in PSUM by triangular-counting matmuls. All matmuls use the
float32r dtype view (full-rate fp32 at free-dim >= 256).
"""

import numpy as np

import concourse.bass as bass
import concourse.mybir as mybir
import concourse.tile as tile
from concourse.bass import ds, ts

F32 = mybir.dt.float32
F32R = mybir.dt.float32r
BF16 = mybir.dt.bfloat16
AF = mybir.ActivationFunctionType

B, T, NE = 2, 2048, 1024
NH, NKV, HD = 16, 4, 64
GC = 32
WIN = 1024
EPS = 1e-6
BIG = 1.0e9
NCORES = 8
QB = 256          # q-block (per head; 2 heads -> 512 moving free dim)
NQB = T // QB     # 8
NKB = T // 128    # 16 k-blocks
SCALE = 1.0 / 8.0  # 1/sqrt(HD)


def _r(x):
    return x.bitcast(F32R)


def _dma_r(nc, dst, src):
    nc.sync.dma_start(_r(dst), _r(src))


def _build_nc():
    nc = bass.Bass(trn_type="TRN2", target_bir_lowering=False)

    d = {}
    for name, shape in [
        ("xT", (NE, T)), ("ve", (128, NKB * HD)),
        ("cos4", (128, T)), ("sin4", (128, T)),
        ("coskv", (128, T)), ("sinkv", (128, T)),
        ("wq", (NE, 256)), ("wkv", (NE, 128)), ("wg", (GC, 1)),
        ("wo", (256, NE)),
        ("pswq", (128, 128)),
        ("bdq", (128, 2)), ("bdk", (128, 1)),
        ("e2sel", (2, 128)), ("ident", (128, 128)),
        ("onesrow", (1, T)),
    ]:
        d[name] = nc.dram_tensor(name, list(shape), F32, kind="ExternalInput")
    for name, shape in [
        ("triA", (128, 128)), ("triA2", (128, 128)),
        ("bc0", (128, 512)), ("bc1", (128, 512)),
        ("bw0", (128, 512)), ("bw1", (128, 512)),
        ("onesv", (128, 64)),
    ]:
        d[name] = nc.dram_tensor(name, list(shape), BF16, kind="ExternalInput")
    out_d = nc.dram_tensor("out", [T, NE], F32, kind="ExternalOutput")

    with tile.TileContext(nc) as tc:
        with (
            nc.allow_low_precision(reason="float32r views of fp32 data"),
            tc.tile_pool(name="persist", bufs=1) as pp,
            tc.tile_pool(name="smalls", bufs=4) as sm,
        ):
            # ---- persistent tiles ----
            # qcat[f, h, t]: 64 features x 4 q-heads x T tokens, rms-scaled
            qcat = pp.tile([64, NH // NKV, T], BF16, tag="qcat")
            # khat: k-hat (roped+rms) bf16; vraw: raw v fp32
            khat = pp.tile([64, T], BF16, tag="khat")
            vraw = pp.tile([64, T], F32, tag="vraw")
            # vaug[kb]: [128 tokens, 64 v-features + 64 ones]
            vaug = [pp.tile([128, 128], BF16, tag=f"vaug{k}", name=f"vaug{k}")
                    for k in range(NKB)]
            cst = {}
            for nm, shp in [("e2sel", [2, 128]), ("ident", [128, 128])]:
                cst[nm] = pp.tile(shp, F32, tag=nm, name=nm)
            _dma_r(nc, cst["e2sel"][:], d["e2sel"][:])
            nc.sync.dma_start(cst["ident"][:], d["ident"][:])
            eps_sb = pp.tile([128, 1], F32, tag="eps")
            nc.vector.memset(eps_sb[:], EPS)
            for kb in range(NKB):
                nc.sync.dma_start(vaug[kb][:, HD:128], d["onesv"][:])

            # =================================================================
            # Phase A: projections + rope + rmsnorm + vaug build
            # =================================================================
            with (
                tc.tile_pool(name="xp", bufs=1) as xp,
                tc.tile_pool(name="work", bufs=2) as wk,
                tc.tile_pool(name="trig", bufs=1) as trg,
                tc.tile_pool(name="pj_ps", bufs=2, space="PSUM") as pjp,
                tc.tile_pool(name="sw_ps", bufs=2, space="PSUM") as swp,
                tc.tile_pool(name="aux_ps", bufs=1, space="PSUM") as axp,
            ):
                xsb = [xp.tile([128, T], F32, tag=f"x{e}", name=f"x{e}") for e in range(8)]
                wq_sb = [xp.tile([128, 256], F32, tag=f"wq{e}", name=f"wqs{e}")
                         for e in range(8)]
                wkv_sb = [xp.tile([128, 128], F32, tag=f"wkv{e}", name=f"wkvs{e}")
                          for e in range(8)]
                for e in range(8):
                    _dma_r(nc, xsb[e][:], d["xT"][ds(128 * e, 128), :])
                    _dma_r(nc, wq_sb[e][:], d["wq"][ds(128 * e, 128), :])
                    _dma_r(nc, wkv_sb[e][:], d["wkv"][ds(128 * e, 128), :])
                wg_sb = sm.tile([GC, 1], F32, tag="wg")
                _dma_r(nc, wg_sb[:], d["wg"][:])
                ve_sb = xp.tile([128, NKB, HD], F32, tag="ve")
                _dma_r(nc, ve_sb[:], d["ve"][:, :])
                aux = {}
                for nm, shp in [("pswq", [128, 128]),
                                ("bdq", [128, 2]), ("bdk", [128, 1])]:
                    aux[nm] = xp.tile(shp, F32, tag=nm, name=f"aux_{nm}")
                    _dma_r(nc, aux[nm][:], d[nm][:])

                # gate: u = x[:, :GC] @ wg ; g2 = 2*sigmoid(u) token-major
                gate_ps = axp.tile([128, NKB], F32, tag="aux")
                for kb in range(NKB):
                    nc.tensor.matmul(
                        gate_ps[:, ds(kb, 1)],
                        xsb[0][0:GC, ts(kb, 128)], wg_sb[:],
                        start=True, stop=True)
                g2 = xp.tile([128, NKB], F32, tag="g2")
                nc.scalar.activation(g2[:], gate_ps[:], AF.Exp, scale=-1.0)
                nc.vector.tensor_scalar_add(g2[:], g2[:], 1.0)
                nc.vector.reciprocal(g2[:], g2[:])
                nc.scalar.mul(g2[:], g2[:], 2.0)

                def proj_part1(widx, w_tiles, mcols, psw, cos_t, sin_t,
                               bd, nh):
                    """x @ W -> roped [128, T] (in place) + rms stats in PSUM."""
                    raw = wk.tile([128, T], F32, tag="w0")
                    for nchk in range(4):
                        cols = ds(512 * nchk, 512)
                        ps = pjp.tile([128, 512], F32, tag="pj")
                        for e in range(8):
                            nc.tensor.matmul(
                                ps[:], _r(w_tiles[e][:, mcols]),
                                _r(xsb[e][:, cols]),
                                start=(e == 0), stop=(e == 7))
                        nc.any.tensor_copy(_r(raw[:, cols]), ps[:])
                    msps = axp.tile([nh, T], F32, tag="aux")
                    for nchk in range(4):
                        cols = ds(512 * nchk, 512)
                        sw = swp.tile([128, 512], F32, tag="sw")
                        nc.tensor.matmul(sw[:], _r(psw[:]), _r(raw[:, cols]),
                                         start=True, stop=True)
                        t1c = wk.tile([128, 512], F32, tag="w1")
                        nc.vector.tensor_mul(_r(t1c[:]), raw[:, cols],
                                             cos_t[:, cols])
                        tm2 = wk.tile([128, 512], F32, tag="w2")
                        nc.vector.tensor_mul(tm2[:], sw[:], sin_t[:, cols])
                        nc.vector.tensor_add(_r(raw[:, cols]), t1c[:], tm2[:])
                        sqc = wk.tile([128, 512], F32, tag="w3")
                        nc.scalar.activation(_r(sqc[:]), raw[:, cols],
                                             AF.Square)
                        nc.tensor.matmul(msps[:, cols], _r(bd[:, 0:nh]),
                                         _r(sqc[:]), start=True, stop=True)
                    return raw, msps

                def proj_rs(widx, msps, nh):
                    """rsqrt(mean-sq) -> rs [2, T] (row 1 stays 1 for nh=1)."""
                    lnm = sm.tile([2, T], F32, tag="lnm", bufs=1,
                                  name=f"lnm{widx}")
                    nc.scalar.activation(lnm[0:nh, :], msps[:], AF.Ln,
                                         scale=1.0 / HD, bias=eps_sb[0:nh, :])
                    rs = sm.tile([2, T], F32, tag="rs", bufs=2,
                                 name=f"rs{widx}")
                    if nh == 1:
                        _dma_r(nc, rs[0:2, :],
                               d["onesrow"][:, :].to_broadcast([2, T]))
                    nc.scalar.activation(_r(rs[0:nh, :]), lnm[0:nh, :], AF.Exp,
                                         scale=-0.5)
                    return rs

                def rsb_chunk(rs, nchk):
                    cols = ds(512 * nchk, 512)
                    rsb = swp.tile([128, 512], F32, tag="sw")
                    nc.tensor.matmul(rsb[:], _r(cst["e2sel"][:]),
                                     _r(rs[0:2, cols]), start=True, stop=True)
                    return rsb

                # --- part1 of kv, then q0 (keeps PE busy during kv stats) ---
                cos_kv = trg.tile([128, T], F32, tag="tc")
                sin_kv = trg.tile([128, T], F32, tag="tsn")
                nc.sync.dma_start(cos_kv[:], d["coskv"][:])
                nc.sync.dma_start(sin_kv[:], d["sinkv"][:])
                ropedkv, msps_kv = proj_part1(2, wkv_sb, ds(0, 128),
                                              aux["pswq"], cos_kv, sin_kv,
                                              aux["bdk"], 1)
                cos_q = trg.tile([128, T], F32, tag="tcq")
                sin_q = trg.tile([128, T], F32, tag="tsnq")
                nc.sync.dma_start(cos_q[:], d["cos4"][:])
                nc.sync.dma_start(sin_q[:], d["sin4"][:])
                roped_q = [None, None]
                roped_q[0], msps_q0 = proj_part1(0, wq_sb, ds(0, 128),
                                                 aux["pswq"], cos_q, sin_q,
                                                 aux["bdq"], 2)
                # --- kv finish: khat (bf16) + vraw ---
                rs_kv = proj_rs(2, msps_kv, 1)
                for nchk in range(4):
                    cols = ds(512 * nchk, 512)
                    rsb = rsb_chunk(rs_kv, nchk)
                    nc.vector.tensor_mul(khat[:, cols], ropedkv[0:64, cols],
                                         rsb[0:64, :])
                    nc.vector.tensor_mul(_r(vraw[:, cols]),
                                         ropedkv[64:128, cols],
                                         rsb[64:128, :])
                # --- q1 part1, then q finishes -> qcat (bf16) ---
                roped_q[1], msps_q1 = proj_part1(1, wq_sb, ds(128, 128),
                                                 aux["pswq"], cos_q, sin_q,
                                                 aux["bdq"], 2)
                for i, msps_qi in ((0, msps_q0), (1, msps_q1)):
                    rs = proj_rs(i, msps_qi, 2)
                    for nchk in range(4):
                        cols = ds(512 * nchk, 512)
                        rsb = rsb_chunk(rs, nchk)
                        for h in range(2):
                            rows = ds(64 * h, 64)
                            nc.vector.tensor_mul(
                                qcat[:, 2 * i + h, cols],
                                roped_q[i][rows, cols], rsb[rows, :])
                # --- vaug build ---
                for kb in range(NKB):
                    vt = pjp.tile([128, HD], F32, tag="pj")
                    nc.tensor.transpose(vt[:], vraw[:, ts(kb, 128)],
                                        cst["ident"][0:64, 0:64])
                    gv = sm.tile([128, HD], F32, tag="gv")
                    nc.vector.tensor_scalar_mul(gv[:], ve_sb[:, kb, :],
                                                g2[:, ds(kb, 1)])
                    nc.vector.tensor_add(vaug[kb][:, 0:HD], gv[:], vt[:])

            # =================================================================
            # Phase B: attention + output projection, software-pipelined:
            # block qb emits QK+exp(qb) interleaved with PV(qb-1), then
            # recip/normalize/outproj(qb-1). The in-order PE never waits on
            # ScalarE exp: PV and outproj of the previous block fill the
            # stream while exp of this block lags behind QK.
            # =================================================================
            with (
                tc.tile_pool(name="pb", bufs=1) as pb,
                tc.tile_pool(name="sc_ps", bufs=2, space="PSUM") as scp,
                tc.tile_pool(name="yt_ps", bufs=1, space="PSUM") as ytp,
                tc.tile_pool(name="po_ps", bufs=2, space="PSUM") as pop,
                tc.tile_pool(name="et", bufs=13) as etp,
                tc.tile_pool(name="ri", bufs=2) as rip,
                tc.tile_pool(name="stage", bufs=4) as stg,
            ):
                ytall = [pb.tile([128, T], F32, tag=f"ytall{i}",
                                 name=f"ytall{i}") for i in range(2)]
                wo_sb = [[pb.tile([128, 512], F32, tag=f"wo{i}{n}",
                                  name=f"wo{i}{n}")
                          for n in range(2)] for i in range(2)]
                for nm in ("triA", "triA2", "bc0", "bc1", "bw0", "bw1"):
                    shp = [128, 128] if nm.startswith("tri") else [128, 512]
                    cst[nm] = pb.tile(shp, BF16, tag=nm, name=f"pb_{nm}")
                    nc.sync.dma_start(cst[nm][:], d[nm][:])
                for i in range(2):
                    for n in range(2):
                        _dma_r(nc, wo_sb[i][n][:],
                               d["wo"][ds(128 * i, 128), ds(512 * n, 512)])

                def kbs_of(qb):
                    return list(range(max(0, 2 * qb - 8), 2 * qb + 2))

                ets = {}       # (qb, kb) -> et tile
                yts_cur = [None]  # yts tile of the in-flight PV block

                def emit_qk2(qb, kbA, kbB):
                    """QK + masks + exp for a pair of k-blocks. Stationaries
                    (khat slice, tri matrix) are loaded once and reused by
                    setting ldweights=False on the following matmuls."""
                    if kbA == 2 * qb:
                        masks = (cst["triA"], (cst["bc0"], cst["bc1"]))
                    elif kbA == 2 * qb - 8:
                        masks = (cst["triA2"], (cst["bw0"], cst["bw1"]))
                    else:
                        masks = None
                    scs = {}
                    for kb in (kbA, kbB):
                        sc = scp.tile([128, 1024], F32, tag="score",
                                      name=f"sc{qb}_{kb}")
                        scs[kb] = sc
                        nc.tensor.ldweights(khat[:, ts(kb, 128)])
                        for pair in range(2):
                            mm = nc.tensor.matmul(
                                sc[:, ds(512 * pair, 512)],
                                khat[:, ts(kb, 128)],
                                qcat[:, ds(2 * pair, 2), ds(QB * qb, QB)],
                                start=True, stop=(masks is None))
                            mm.ins.ldweights = False
                    if masks is not None:
                        tri, bcs = masks
                        nc.tensor.ldweights(tri[:])
                        for kb, bc in zip((kbA, kbB), bcs):
                            for pair in range(2):
                                mm = nc.tensor.matmul(
                                    scs[kb][:, ds(512 * pair, 512)],
                                    tri[:], bc[:], start=False, stop=True)
                                mm.ins.ldweights = False
                    for kb in (kbA, kbB):
                        et = etp.tile([128, 1024], BF16, tag="et",
                                      name=f"et{qb}_{kb}")
                        nc.scalar.activation(et[:], scs[kb][:], AF.Exp,
                                             scale=SCALE)
                        ets[(qb, kb)] = et

                def emit_pv2(qb, kbA, kbB, first, last):
                    if first:
                        yts_cur[0] = ytp.tile([128, 1024], F32, tag="yt",
                                              name=f"yt{qb}")
                    for j, kb in enumerate((kbA, kbB)):
                        et = ets.pop((qb, kb))
                        nc.tensor.ldweights(vaug[kb][:])
                        for pair in range(2):
                            mm = nc.tensor.matmul(
                                yts_cur[0][:, ds(512 * pair, 512)],
                                vaug[kb][:], et[:, ds(512 * pair, 512)],
                                start=first and j == 0,
                                stop=last and j == 1)
                            mm.ins.ldweights = False

                def emit_recip(qb):
                    """1/denominator via exp(-ln d) on ScalarE."""
                    yts = yts_cur[0]
                    lnd = rip.tile([64, 1024], F32, tag="lnd")
                    nc.scalar.activation(lnd[:], yts[64:128, :], AF.Ln)
                    rinv = rip.tile([64, 1024], F32, tag="ri")
                    nc.scalar.activation(_r(rinv[:]), lnd[:], AF.Exp,
                                         scale=-1.0)
                    return yts, rinv

                def emit_norm(qb, yts, rinv):
                    qsl = ds(QB * qb, QB)
                    for pair in range(2):
                        for h in range(2):
                            nc.vector.tensor_mul(
                                _r(ytall[pair][ds(64 * h, 64), qsl]),
                                yts[0:HD, ds(512 * pair + QB * h, QB)],
                                rinv[:, ds(512 * pair + QB * h, QB)])

                def emit_outproj(tt):
                    for nn in range(2):
                        po = pop.tile([128, 512], F32, tag="po")
                        for i in range(2):
                            nc.tensor.matmul(
                                po[:], _r(ytall[i][:, ts(tt, 128)]),
                                _r(wo_sb[i][nn][:]),
                                start=(i == 0), stop=(i == 1))
                        osb = stg.tile([128, 512], F32, tag="osb")
                        nc.vector.tensor_copy(_r(osb[:]), po[:])
                        nc.sync.dma_start(
                            out_d[ts(tt, 128), ds(512 * nn, 512)], osb[:])

                for qb in range(NQB + 1):
                    cur = kbs_of(qb) if qb < NQB else []
                    prev = kbs_of(qb - 1) if qb > 0 else []
                    ng_c, ng_p = len(cur) // 2, len(prev) // 2
                    fin = None
                    for j in range(max(ng_c, ng_p)):
                        if j < ng_p:
                            emit_pv2(qb - 1, prev[2 * j], prev[2 * j + 1],
                                     j == 0, j == ng_p - 1)
                            if j == ng_p - 1:
                                fin = emit_recip(qb - 1)
                        if j < ng_c:
                            emit_qk2(qb, cur[2 * j], cur[2 * j + 1])
                    if fin is not None:
                        yts_p, rinv_p = fin
                        emit_norm(qb - 1, yts_p, rinv_p)
                        emit_outproj(2 * (qb - 1))
                        emit_outproj(2 * (qb - 1) + 1)

    return nc


# ---------------------------------------------------------------------------
# walrus workaround: this build rejects >1 sync-wait on CTRL-class ops
# (e.g. the Tile tail Drain). Move excess waits onto NOPs inserted before.
# ---------------------------------------------------------------------------
_CTRL_TYPES = (mybir.InstDrain, mybir.InstNoOp, mybir.InstEventSemaphore)


def _split_excess_waits(nc, limit=1):
    for fn in nc.m.functions:
        for bb in fn.blocks:
            out, changed = [], False
            for inst in bb.instructions:
                si = inst.sync_info
                waits = list(si.on_wait) if si is not None and si.on_wait else []
                if len(waits) > limit:
                    extra, keep = waits[:-limit], waits[-limit:]
                    while extra:
                        chunk, extra = extra[:limit], extra[limit:]
                        nop = mybir.InstNoOp(
                            name=f"{inst.name}-wsplit{len(out)}", ins=[],
                            outs=[])
                        nop.engine = inst.engine
                        nop.sync_info = mybir.SyncInfo(on_wait=chunk,
                                                       on_update=[])
                        out.append(nop)
                    si.on_wait = keep
                    inst.sync_info = si
                    changed = True
                out.append(inst)
            if changed:
                bb.instructions = out


# ---------------------------------------------------------------------------
# Host-side constants (shared by all cores)
# ---------------------------------------------------------------------------
def _host_constants():
    import ml_dtypes
    bf16 = ml_dtypes.bfloat16
    c = {}
    m = np.arange(128)[:, None]
    i = np.arange(QB)[None, :]
    c["triA"] = (m <= np.arange(128)[None, :]).astype(bf16)
    c["triA2"] = (m >= np.arange(128)[None, :]).astype(bf16)
    bc0 = np.where(m > i, -BIG, 0.0).astype(np.float32)
    bc1 = np.where(m > i - 128, -BIG, 0.0).astype(np.float32)
    bw0 = np.where(m < i, -BIG, 0.0).astype(np.float32)
    bw1 = np.where(m + 128 < i, -BIG, 0.0).astype(np.float32)
    for nm, base in (("bc0", bc0), ("bc1", bc1), ("bw0", bw0), ("bw1", bw1)):
        c[nm] = np.tile(base, (1, 2)).astype(bf16)
    sw = np.zeros((128, 128), np.float32)            # pswq[f, m]=1 iff f=sig(m)
    for mm in range(128):
        f = mm + 32 if (mm % 64) < 32 else mm - 32
        sw[f, mm] = 1.0
    c["pswq"] = sw
    bdq = np.zeros((128, 2), np.float32)
    bdq[0:64, 0] = 1.0
    bdq[64:128, 1] = 1.0
    c["bdq"] = bdq
    bdk = np.zeros((128, 1), np.float32)
    bdk[0:64, 0] = 1.0
    c["bdk"] = bdk
    e2 = np.zeros((2, 128), np.float32)
    e2[0, 0:64] = 1.0
    e2[1, 64:128] = 1.0
    c["e2sel"] = e2
    c["ident"] = np.eye(128, dtype=np.float32)
    c["onesrow"] = np.ones((1, T), np.float32)
    c["onesv"] = np.ones((128, 64), bf16)
    return c


def _trig(cos_b, sin_b):
    """cos_b/sin_b: [T, HD//2] -> the four [128, T] rope coefficient maps."""
    ct = np.ascontiguousarray(cos_b.T)               # [32, T]
    st = np.ascontiguousarray(sin_b.T)
    cos4 = np.tile(ct, (4, 1)).astype(np.float32)    # [c;c;c;c]
    sin4 = np.tile(np.concatenate([st, -st], 0), (2, 1)).astype(np.float32)
    coskv = np.concatenate([ct, ct, np.ones((64, T), np.float32)], 0)
    sinkv = np.concatenate([st, -st, np.zeros((64, T), np.float32)], 0)
    return cos4, sin4, coskv.astype(np.float32), sinkv.astype(np.float32)


# ---------------------------------------------------------------------------
# Cached PJRT runner (compile once per process)
# ---------------------------------------------------------------------------
_RUNNER = None


def _get_runner():
    global _RUNNER
    if _RUNNER is not None:
        return _RUNNER
    import jax
    from jax.experimental.shard_map import shard_map
    from jax.sharding import Mesh, PartitionSpec
    from concourse.bass2jax import (_bass_exec_p, install_neuronx_cc_hook,
                                    partition_id_tensor)

    nc = _build_nc()
    _split_excess_waits(nc)
    install_neuronx_cc_hook()

    pid_name = (nc.partition_id_tensor.name
                if nc.partition_id_tensor is not None else None)
    in_names, out_names, out_avals, zero_outs = [], [], [], []
    for alloc in nc.m.functions[0].allocations:
        if not isinstance(alloc, mybir.MemoryLocationSet):
            continue
        name = alloc.memorylocations[0].name
        if alloc.kind == "ExternalInput":
            if name == pid_name:
                continue
            in_names.append(name)
        elif alloc.kind == "ExternalOutput":
            np_dt = mybir.dt.np(alloc.dtype)
            out_names.append(name)
            out_avals.append(
                jax.core.ShapedArray(tuple(alloc.tensor_shape), np_dt))
            zero_outs.append(
                np.zeros(tuple(alloc.tensor_shape), np_dt))

    def _body(*args):
        operands = list(args)
        if pid_name is not None:
            operands.append(partition_id_tensor())
        outs = _bass_exec_p.bind(
            *operands,
            out_avals=tuple(out_avals),
            in_names=(tuple(in_names) + tuple(out_names)
                      + ((pid_name,) if pid_name else ())),
            out_names=tuple(out_names),
            lowering_input_output_aliases=(),
            sim_require_finite=True,
            sim_require_nnan=True,
            nc=nc,
        )
        return tuple(outs)

    devices = jax.devices()[:NCORES]
    mesh = Mesh(np.asarray(devices), ("core",))
    n_args = len(in_names) + len(out_names)
    sharded = jax.jit(
        shard_map(_body, mesh=mesh,
                  in_specs=(PartitionSpec("core"),) * n_args,
                  out_specs=(PartitionSpec("core"),) * len(out_names),
                  check_rep=False),
        keep_unused=True,
    )

    def run(in_maps):
        concat_in = [
            np.concatenate([in_maps[c][nm] for c in range(NCORES)], axis=0)
            for nm in in_names
        ]
        concat_zero = [
            np.zeros((NCORES * z.shape[0], *z.shape[1:]), z.dtype)
            for z in zero_outs
        ]
        outs = sharded(*concat_in, *concat_zero)
        res = []
        for c in range(NCORES):
            res.append({
                nm: np.asarray(outs[i]).reshape(NCORES, *out_avals[i].shape)[c]
                for i, nm in enumerate(out_names)
            })
        return res

    _RUNNER = {"run": run, "sharded": sharded, "in_names": in_names,
               "out_names": out_names, "out_avals": out_avals,
               "zero_outs": zero_outs, "nc": nc, "mesh": mesh}
    return _RUNNER


def _make_in_maps(x, ve, cos, sin, Wq, Wk, Wv, Wo, Wg):
    cstc = _host_constants()
    in_maps = []
    for c in range(NCORES):
        b, g = c // 4, c % 4
        cos4, sin4, coskv, sinkv = _trig(np.asarray(cos[b]),
                                         np.asarray(sin[b]))
        m = {
            "xT": np.ascontiguousarray(np.asarray(x[b]).T),
            "ve": np.ascontiguousarray(
                np.asarray(ve[b])[:, HD * g:HD * (g + 1)]
                .reshape(NKB, 128, HD).transpose(1, 0, 2).reshape(128, -1)),
            "cos4": cos4, "sin4": sin4, "coskv": coskv, "sinkv": sinkv,
            "wq": np.ascontiguousarray(Wq[:, 256 * g:256 * (g + 1)]),
            "wkv": np.ascontiguousarray(
                np.concatenate([Wk[:, HD * g:HD * (g + 1)],
                                Wv[:, HD * g:HD * (g + 1)]], axis=1)),
            "wg": np.ascontiguousarray(Wg[:, g:g + 1]),
            "wo": np.ascontiguousarray(Wo[256 * g:256 * (g + 1), :]),
        }
        m = {k: np.asarray(v, np.float32) for k, v in m.items()}
        m.update(cstc)
        in_maps.append(m)
    return in_maps


def kernel(x, ve, cos, sin, Wq, Wk, Wv, Wo, Wg, window_size):
    assert int(window_size) == WIN, f"kernel hardcodes window={WIN}"
    x, ve, cos, sin = (np.asarray(a, np.float32) for a in (x, ve, cos, sin))
    Wq, Wk, Wv, Wo, Wg = (np.asarray(a, np.float32)
                          for a in (Wq, Wk, Wv, Wo, Wg))
    runner = _get_runner()
    in_maps = _make_in_maps(x, ve, cos, sin, Wq, Wk, Wv, Wo, Wg)
    res = runner["run"](in_maps)
    out = np.zeros((B, T, NE), np.float32)
    for c in range(NCORES):
        out[c // 4] += res[c]["out"]
    return out
